# revision 26
# baseline (speedup 1.0000x reference)
"""Trainium2 Bass kernel for nn_Net_25847113187867 (dense_cnn).

The reference slides W = 16384 stride-1 windows over x (1,2,L), runs
conv(s5)/conv(s3)/conv(s2) + 3-layer MLP + hidden-size-1 Elman RNN per
window, twice (second pass with x channel 0 negated), and returns the
antisymmetrized scan outputs (y - y_)/2.

Restructure (final):
  * Window conv stack == dilated convs over the full sequence; fc3+RNN
    input row folded into one 80->1 vector on the host; conv1 bias
    folded into an ones-row of the input matrix.
  * Pass A and pass B (negated ch0) share one activation tile per conv
    stage ([A; B] across the 128 partitions); conv2/conv3 use
    block-diagonal weights so one matmul per tap computes both passes.
  * conv2/conv3/fc1 run fully in bf16 (weights shipped bf16 and used
    directly -- no on-device widening; activations evacuated to bf16),
    halving LDWEIGHTS time and evacuation cost.  fc2/xp stay f32r.
    Total dtype error ~8e-3 vs the 2e-2 gate.
  * Startup: SBUF DMA writes are partition-port-bound (~2GB/s per
    partition), so the 11-row conv1 window matrix ships in bf16 as 3
    column-thirds, one per DMA queue, at partition offsets 0/32/64 (PE
    weight tiles at 32-aligned rows).  conv1 evacs split across both
    act engines at an aligned 512 boundary so conv2 unblocks early.
    5 fp32 warm-up matmuls ramp the HAM clock gate while the input
    lands; 1 filler bridges c1->c2 (a PE gap drops the clock p-state).
  * fc2/xp interleave into fc1's window (same column-group of fc1
    feeds them); per pass the 3 fc2 chains run back-to-back, then the
    3 xp groups -- evacuations hide under later chains.  fc2
    accumulation steps interleave the two psum sub-blocks.
  * xp matmuls write 2 identical psum rows (VV has 2 columns), so one
    [2,gw] store lands 2 XPR copies on adjacent partitions = 2
    independent DMA-gather read ports; stores split across vector and
    scalar so gathers wait on at most 2 writer engines.
  * Chunked tanh scan: 63 rows x 33 outputs per pass, 20-step warmup
    halo, 2 DEER/Newton iterations seeded at h=0 (the seed quality is
    irrelevant after 2 iterations), so iteration 1 degenerates to
    tanh+Jacobian+scan (6 ops).  Degree-3 tanh polynomial everywhere
    (|z|<=0.22; errors correlate across the antisymmetrized passes).
    Pass-A iteration 1 runs while pass B is still on the PE; the final
    combine defers the /2 to the host.
  * Matmul column blocks all >= 256 (f32r runs 4x slower below 256),
    sub-blocks <= 512 (psum-bank limit on the matmul dst).
  * 8 cores split outputs into 2048-position slices (overlapping input
    halos, no collectives).  85.9us baseline -> 69.7us.
"""

import numpy as np

L = 16684
W = 16384
P = 2048            # output positions per core
CH = 33             # scan chunk length (output steps per chunk row)
KW = 20             # per-chunk warmup halo steps (|whh|^20 * 0.33 ~ 2e-3)
SC = KW + CH        # 53 scan columns per chunk row
HALO = KW           # left halo of xp positions per core
NY = 62 * CH + SC + 1  # 2100 xp positions per core: [s-20, s+2080)
NC3 = NY + 204      # 2304 c3 positions per core (>= NY+180, padded so
                    # _groups(NC3, balance=True) hits the [1024,x,1024] form)
NC2 = NC3 + 76      # 2380
NC1 = NC2 + 26      # 2406
NX = NC1 + 6        # 2412
SCAN_ITERS = 2
N_WARMUP = 5        # dummy fp32 matmuls to ramp the PE clock gate


def _groups(n, balance=False):
    """column groups (<=1024, psum-bank pair) with all sub-blocks in
    [256, 512], never crossing a 512-col psum bank line, everything even.
    balance=True puts the small remainder group in the MIDDLE so the
    stage ends with a full-length group whose matmuls cover the earlier
    groups' evacuation latency."""
    assert n % 2 == 0
    if balance and 2048 + 256 <= n <= 3072:
        widths = [1024, n - 2048, 1024]
    else:
        widths = []
        o = 0
        while o < n:
            rem = n - o
            gw = min(1024, rem)
            if rem > gw and rem - gw < 256:
                gw = rem - 256
            widths.append(gw)
            o += gw
    out, o = [], 0
    for gw in widths:
        if gw <= 512:
            subs = [(0, gw)]
        elif gw <= 768:
            subs = [(0, 512), (512, gw - 512)] if gw - 512 >= 256 else \
                   [(0, gw - 256), (gw - 256, 256)]
        else:
            subs = [(0, 512), (512, gw - 512)]
        out.append((o, gw, subs))
        o += gw
    return out


def _build_program(a_const, c0_const):
    import concourse.bass as bass
    import concourse.mybir as mybir
    import concourse.tile as tile
    from concourse import bacc
    from contextlib import ExitStack

    dt = mybir.dt
    f32 = dt.float32
    AF = mybir.ActivationFunctionType
    OP = mybir.AluOpType
    f32r = dt.float32r
    bf16 = dt.bfloat16

    C2 = 160 + 80 + 2              # PK2 cols (f32r): F2 F2S2 VV(pad 2)
    C3 = 2 + 3 + 1 + SC + 1        # PK3 cols: B2AB B3AB FB1 FB2 MASK (+pad)
    QW = 810                       # conv1 input cols per third (802+halo)
    QC = 128 + QW                  # third cols: W1 then windowed input

    nc = bacc.Bacc("TRN2", target_bir_lowering=False, debug=False,
                   num_devices=8)

    xq0_d = nc.dram_tensor("xq0", [11, QC], bf16, kind="ExternalInput")
    xq1_d = nc.dram_tensor("xq1", [11, QC], bf16, kind="ExternalInput")
    xq2_d = nc.dram_tensor("xq2", [11, QC], bf16, kind="ExternalInput")
    pw2_d = nc.dram_tensor("pw2", [128, 768], bf16, kind="ExternalInput")
    pw3_d = nc.dram_tensor("pw3", [128, 768], bf16, kind="ExternalInput")
    pkb_d = nc.dram_tensor("pkb", [128, 960], bf16, kind="ExternalInput")
    pks_d = nc.dram_tensor("pks", [64, 320], bf16, kind="ExternalInput")
    pkx_d = nc.dram_tensor("pkx", [128, 128], bf16, kind="ExternalInput")
    pk2_d = nc.dram_tensor("pk2", [128, C2], f32r, kind="ExternalInput")
    pk3_d = nc.dram_tensor("pk3", [128, C3], f32, kind="ExternalInput")
    y_d = nc.dram_tensor("y", [1, 63 * CH], f32, kind="ExternalOutput")

    with ExitStack() as ctx:
        tc = ctx.enter_context(tile.TileContext(nc))
        wp = ctx.enter_context(tc.tile_pool(name="weights", bufs=1))
        sp = ctx.enter_context(tc.tile_pool(name="acts", bufs=1))
        pp = ctx.enter_context(tc.tile_pool(name="ps", bufs=4, space="PSUM"))

        WU = wp.tile([128, 192], f32, name="WU", tag="WU")
        XWQ = wp.tile([75, QC], bf16, name="xwq", tag="xwq")
        W2B = wp.tile([128, 768], bf16, name="pw2", tag="pw2")
        W3B = wp.tile([128, 768], bf16, name="pw3", tag="pw3")
        F1P = wp.tile([128, 960], bf16, name="pkb", tag="pkb")
        F1S = wp.tile([64, 320], bf16, name="pks", tag="pks")
        F1X = wp.tile([128, 128], bf16, name="pkx", tag="pkx")
        PK2 = wp.tile([128, C2], f32r, name="pk2", tag="pk2")
        PK3 = wp.tile([128, C3], f32, name="pk3", tag="pk3")

        # -------- DMA issue: one input third per queue, then weights ----
        nc.vector.memset(WU[:, :], 0.0)
        nc.sync.dma_start(XWQ[0:11, :], xq0_d.ap())
        nc.scalar.dma_start(XWQ[32:43, :], xq1_d.ap())
        nc.gpsimd.dma_start(XWQ[64:75, :], xq2_d.ap())
        nc.sync.dma_start(W2B[:, 0:384], pw2_d.ap()[:, 0:384])
        nc.scalar.dma_start(W2B[:, 384:768], pw2_d.ap()[:, 384:768])
        nc.sync.dma_start(PK3[:], pk3_d.ap())
        nc.sync.dma_start(W3B[:], pw3_d.ap())
        nc.gpsimd.dma_start(F1P[:], pkb_d.ap())
        nc.sync.dma_start(PK2[:], pk2_d.ap())
        nc.sync.dma_start(F1S[:], pks_d.ap())
        nc.sync.dma_start(F1X[:], pkx_d.ap())

        F2 = PK2[:, 0:160]
        F2S2 = PK2[:, 160:240]
        VV = PK2[:, 240:242]
        B2AB = PK3[:, 0:1]
        B3AB = PK3[:, 1:2]
        FB1 = PK3[:, 2:5]
        FB2 = PK3[:, 5:6]
        MASK = PK3[:, 6:6 + SC]

        SAB = sp.tile([128, NC1], bf16, name="SAB", tag="SAB")
        TAB = sp.tile([128, NC2], bf16, name="TAB", tag="TAB")
        U_ = {"A": sp.tile([128, NC3], bf16, name="UA", tag="UA"),
              "B": sp.tile([128, NC3], bf16, name="UB", tag="UB")}
        UAB = sp.tile([128, NC3], bf16, name="UAB", tag="UAB")
        Y1 = {("A", 0): sp.tile([128, NY], f32r, name="Y1A0", tag="Y1A0"),
              ("A", 1): sp.tile([128, NY], f32r, name="Y1A1", tag="Y1A1"),
              ("B", 0): sp.tile([128, NY], f32r, name="Y1B0", tag="Y1B0"),
              ("B", 1): sp.tile([128, NY], f32r, name="Y1B1", tag="Y1B1")}
        Y12 = sp.tile([128, NY], f32r, name="Y12", tag="Y12")
        Y2 = {"A": sp.tile([80, NY], f32r, name="Y2A", tag="Y2A"),
              "B": sp.tile([80, NY], f32r, name="Y2B", tag="Y2B")}
        XP2 = sp.tile([34, NY], f32, name="XPR", tag="XPR")
        XPR = {"A": XP2[0:2, :], "B": XP2[32:34, :]}

        # all scan scratch lives in one tile; views below carve it up
        SS = sp.tile([128, 6 * SC + 2 * (SC + 1) + 2 * CH], f32,
                     name="SS", tag="SS")
        SCT = SS[:, 0 * SC:1 * SC]
        ZT = SS[:, 1 * SC:2 * SC]
        FT = SS[:, 2 * SC:3 * SC]
        GT = SS[:, 3 * SC:4 * SC]
        DT = SS[:, 4 * SC:5 * SC]
        T2T = SS[:, 5 * SC:6 * SC]
        # cur tiles carry a zero leading column: view [:, 1:SC+1] is the
        # value, [:, 0:SC] is the shifted-by-one view
        HT = SS[:, 6 * SC:7 * SC + 1]
        H2T = SS[:, 7 * SC + 1:8 * SC + 2]
        CB = SS[:, 8 * SC + 2:8 * SC + 2 + CH]
        D = SS[:, 8 * SC + 2 + CH:8 * SC + 2 + 2 * CH]

        # zero the leading columns of the cur tiles + garbage scan rows
        nc.vector.memset(H2T[:, 0:1], 0.0)
        nc.vector.memset(SCT[0:1, :], 0.0)
        nc.vector.memset(SCT[64:65, :], 0.0)

        # ---------------- warm-up (ramps HAM clock gate) ----------------
        for i in range(N_WARMUP):
            pw = pp.tile([128, 1024], f32, name="ps", tag="ps")
            nc.tensor.matmul(pw[0:32, 0:160], WU[:, 160:192], WU[:, 0:160],
                             start=True, stop=True)

        _ct = [0]

        def evac(out_ap, ps_ap, bias_ap, eng=None):
            """relu(ps + bias) -> out; alternate scalar / vector engines."""
            if eng is None:
                use_act = _ct[0] % 2 == 0
                _ct[0] += 1
            else:
                use_act = eng == "scalar"
            if use_act:
                if bias_ap is None:
                    nc.scalar.activation(out_ap, ps_ap, AF.Relu)
                else:
                    nc.scalar.activation(out_ap, ps_ap, AF.Relu,
                                         bias=bias_ap)
            else:
                if bias_ap is None:
                    nc.vector.tensor_scalar(out_ap, ps_ap, 0.0, None, OP.max)
                else:
                    nc.vector.tensor_scalar(out_ap, ps_ap, bias_ap, 0.0,
                                            OP.add, OP.max)

        # ---------------- c1: both passes in one matmul (bf16) ----------
        # third q (PE tile rows 32q) covers global conv1 sites
        # [802q, 802q+802)
        for q in (0, 1, 2):
            rb = 32 * q
            ps = pp.tile([128, 1024], f32, name="ps", tag="ps")
            for bo, nb in ((0, 512), (512, 290)):
                nc.tensor.matmul(ps[:, bo:bo + nb],
                                 XWQ[rb:rb + 11, 0:128],
                                 XWQ[rb:rb + 11, 128 + bo:128 + bo + nb],
                                 start=True, stop=True)
            # split the evac across both engines so conv2 unblocks sooner
            evac(SAB[:, 802 * q:802 * q + 512], ps[:, 0:512], None,
                 eng="scalar")
            evac(SAB[:, 802 * q + 512:802 * q + 802], ps[:, 512:802], None,
                 eng="vector")

        # fillers: bridge c1 end -> conv2 weight DMA completion
        for i in range(1):
            pw = pp.tile([128, 1024], f32, name="ps", tag="ps")
            nc.tensor.matmul(pw[0:32, 0:160], WU[:, 160:192], WU[:, 0:160],
                             start=True, stop=True)

        # ------------- c2/c3: block-diagonal dilated convs --------------
        def conv_stage(SRC, n_out, Wt, dil):
            for goff, gw, subs in _groups(n_out, balance=True):
                ps = pp.tile([128, 1024], f32, name="ps", tag="ps")
                for t in range(6):
                    for bo, nb in subs:
                        o = goff + bo
                        nc.tensor.matmul(
                            ps[:, bo:bo + nb],
                            Wt[:, 128 * t:128 * t + 128],
                            SRC[:, o + dil * t:o + dil * t + nb],
                            start=(t == 0), stop=(t == 5))
                yield goff, gw, ps

        for goff, gw, ps in conv_stage(SAB, NC2, W2B, 5):
            evac(TAB[:, goff:goff + gw], ps[:, :gw], B2AB)

        # c3 evacuates into per-pass [site; site+30] stacks for fc1, and
        # a combined [A site+180; B site+180] stack so fc1's chunk-2
        # tap-6 step contracts both passes in one 128-deep matmul
        for goff, gw, ps in conv_stage(TAB, NC3, W3B, 15):
            for pX, rows in (("A", slice(0, 64)), ("B", slice(64, 128))):
                dst = U_[pX]
                evac(dst[0:64, goff:goff + gw], ps[rows, :gw],
                     B3AB[rows, :])
                if goff == 0:
                    evac(dst[64:128, 0:gw - 30], ps[rows, 30:gw],
                         B3AB[rows, :])
                else:
                    evac(dst[64:128, goff - 30:goff + gw - 30],
                         ps[rows, :gw], B3AB[rows, :])
                if goff == 0:
                    evac(UAB[rows, 0:gw - 180], ps[rows, 180:gw],
                         B3AB[rows, :])
                else:
                    evac(UAB[rows, goff - 180:goff + gw - 180],
                         ps[rows, :gw], B3AB[rows, :])

        # ---------------- fc1: 448 -> 320 (tap pairs) -------------------
        gl = _groups(NY)

        def fc1_grp(gi):
            goff, gw, subs = gl[gi]
            for c in range(2):
                psc = {pX: pp.tile([128, 1024], f32, name="ps", tag="ps")
                       for pX in "AB"}
                for p in range(3):
                    wap = F1P[:, 320 * p + 128 * c:320 * p + 128 * c + 128]
                    for pX in "AB":
                        U = U_[pX]
                        for bo, nb in subs:
                            o = goff + bo
                            nc.tensor.matmul(
                                psc[pX][:, bo:bo + nb], wap,
                                U[:, o + 60 * p:o + 60 * p + nb],
                                start=(p == 0), stop=False)
                for pX in "AB":
                    U = U_[pX]
                    for bo, nb in subs:
                        o = goff + bo
                        nc.tensor.matmul(
                            psc[pX][:, bo:bo + nb],
                            F1S[0:64, 128 * c:128 * c + 128],
                            U[0:64, o + 180:o + 180 + nb],
                            start=False, stop=True)
                for pX in "AB":
                    evac(Y1[(pX, c)][:, goff:goff + gw], psc[pX][:, :gw],
                         FB1[:, c:c + 1])
            # chunk 2 (64 outs): A -> psum rows 0:64, B -> rows 64:128;
            # the tap-6 step contracts [A; B] in one block-diag matmul
            psc2 = pp.tile([128, 1024], f32, name="ps", tag="ps")
            for p in range(3):
                wap = F1P[:, 320 * p + 256:320 * p + 320]
                for pX, pr in (("A", 0), ("B", 64)):
                    U = U_[pX]
                    for bo, nb in subs:
                        o = goff + bo
                        nc.tensor.matmul(
                            psc2[pr:pr + 64, bo:bo + nb], wap,
                            U[:, o + 60 * p:o + 60 * p + nb],
                            start=(p == 0), stop=False)
            for bo, nb in subs:
                o = goff + bo
                nc.tensor.matmul(
                    psc2[:, bo:bo + nb], F1X[:, :],
                    UAB[:, o:o + nb],
                    start=False, stop=True)
            for pX, pr in (("A", 0), ("B", 64)):
                evac(Y12[pr:pr + 64, goff:goff + gw], psc2[pr:pr + 64, :gw],
                     FB1[pr:pr + 64, 2:3])

        # ------------- fc2 + xp: pass A, then pass B --------------------
        def fc2_mm(pX, gi):
            # accumulation steps interleave the psum sub-blocks so the
            # chains pipeline; one LDW per step serves both sub-blocks.
            goff, gw, subs = gl[gi]
            ps = pp.tile([128, 1024], f32, name="ps", tag="ps")
            pr = 0 if pX == "A" else 64
            steps = ((F2[:, 0:80], Y1[(pX, 0)], slice(0, 128), 0),
                     (F2[:, 80:160], Y1[(pX, 1)], slice(0, 128), 1),
                     (F2S2[pr:pr + 64, :], Y12, slice(pr, pr + 64), 2))
            for wap, src, rows, k in steps:
                for bo, nb in subs:
                    o = goff + bo
                    nc.tensor.matmul(ps[:80, bo:bo + nb], wap,
                                     src[rows, o:o + nb],
                                     start=(k == 0), stop=(k == 2))
            evac(Y2[pX][:, goff:goff + gw], ps[:80, :gw], FB2[0:80, :],
                 eng="scalar")

        def xp_mm(pX, gi):
            # the VV stationary has 2 identical columns -> psum rows 0:2
            # are 2 copies; one [2, gw] store lands both XPR copies (on
            # adjacent partitions = 2 independent DMA-gather read ports)
            goff, gw, subs = gl[gi]
            ps2 = pp.tile([128, 1024], f32, name="ps", tag="ps")
            for bo, nb in subs:
                o = goff + bo
                nc.tensor.matmul(ps2[:2, bo:bo + nb], VV[0:80, :],
                                 Y2[pX][:, o:o + nb], start=True, stop=True)
            # A stores + the tail-critical B g1/g2 stores on vector; B g0
            # on scalar (consecutive same-engine stores keep each gather
            # at <= 2 writer semaphores)
            if pX == "A" or gi == 2:
                nc.vector.tensor_scalar(XPR[pX][0:2, goff:goff + gw],
                                        ps2[:2, :gw], float(c0_const), None,
                                        OP.add)
            else:
                h = (gw // 2) & ~1
                nc.vector.tensor_scalar(XPR[pX][0:2, goff:goff + h],
                                        ps2[:2, :h], float(c0_const), None,
                                        OP.add)
                nc.scalar.activation(XPR[pX][0:2, goff + h:goff + gw],
                                     ps2[:2, h:gw], AF.Copy,
                                     bias=float(c0_const))

        # ------------- merged A/B chunked tanh scan pieces --------------
        # rows 1:64 = pass A chunks, rows 65:128 = pass B chunks
        def ptanh(out, z, rows, deg3=False, eng=None):
            """tanh(z) for |z|<=0.35 as z*(1 - t/3 + 2t^2/15), t=z^2.
            deg3 drops the t^2 term (err ~3e-4): fine for points that
            only seed a Newton step."""
            e = eng or nc.vector
            t2 = T2T[rows, :]
            g = GT[rows, :]
            e.tensor_tensor(t2, z, z, OP.mult)
            if deg3:
                e.tensor_scalar(g, t2, -1.0 / 3.0, 1.0, OP.mult, OP.add)
                e.tensor_tensor(out, g, z, OP.mult)
                return
            e.tensor_scalar(g, t2, 2.0 / 15.0, -1.0 / 3.0, OP.mult, OP.add)
            e.tensor_tensor(g, g, t2, OP.mult)
            e.scalar_tensor_tensor(out, g, 1.0, z, OP.add, OP.mult)

        def gather(pX, r0, r1, eng, cp=0):
            # gather xp chunk rows [r0, r1) of this pass into SCT; cp
            # selects which XPR copy (partition) serves the read
            rlo = 1 if pX == "A" else 65
            xpr = XPR[pX][cp:cp + 1, :]
            src = bass.AP(tensor=xpr.tensor,
                          offset=xpr.offset + CH * r0,
                          ap=[[NY, 1], [CH, r1 - r0], [1, SC]])
            eng.dma_start(SCT[rlo + r0:rlo + r1, :], src)

        def scan_prep(lo, hi):
            # mask only: the scan seeds at h=0 (two Newton iterations wash
            # out the seed entirely; verified 1.9e-4 scan error)
            half = slice(lo, hi)
            nc.vector.tensor_tensor(SCT[half, :], SCT[half, :],
                                    MASK[half, :], OP.mult)

        def scan_iter1(lo, hi):
            # iteration 1 specialized for the zero seed: Z == sct and
            # d1 == F, so it is just tanh, the Jacobian, and the scan
            hf = slice(lo, hi)
            ptanh(FT[hf, :], SCT[hf, :], hf, deg3=True)
            nc.vector.tensor_tensor(GT[hf, :], FT[hf, :], FT[hf, :], OP.mult)
            nc.vector.tensor_scalar(GT[hf, :], GT[hf, :], -a, a, OP.mult,
                                    OP.add)
            nc.vector.tensor_tensor_scan(H2T[hf, 1:SC + 1], GT[hf, :],
                                         FT[hf, :], 0.0, OP.mult, OP.add)

        a = float(a_const)

        def scan_iter(cur, nxt, deg3, lo=0, hi=128, eng=None):
            e = eng or nc.vector
            hf = slice(lo, hi)
            # Z = a*cur_shifted + sct  (leading zero col makes col0 = sct0)
            if e is nc.vector:
                e.scalar_tensor_tensor(ZT[hf, :], cur[hf, 0:SC], a,
                                       SCT[hf, :], OP.mult, OP.add)
            else:
                # gpsimd has no scalar_tensor_tensor
                e.tensor_scalar(ZT[hf, :], cur[hf, 0:SC], a, None, OP.mult)
                e.tensor_tensor(ZT[hf, :], ZT[hf, :], SCT[hf, :], OP.add)
            ptanh(FT[hf, :], ZT[hf, :], hf, deg3=deg3, eng=e)
            # G = a*(1 - F^2)
            e.tensor_tensor(GT[hf, :], FT[hf, :], FT[hf, :], OP.mult)
            e.tensor_scalar(GT[hf, :], GT[hf, :], -a, a, OP.mult,
                            OP.add)
            # d1 = F - G*cur_shifted;  nxt_t = G_t*nxt_{t-1} + d1_t
            e.tensor_tensor(T2T[hf, :], GT[hf, :], cur[hf, 0:SC],
                            OP.mult)
            e.tensor_tensor(DT[hf, :], FT[hf, :], T2T[hf, :],
                            OP.subtract)
            e.tensor_tensor_scan(nxt[hf, 1:SC + 1], GT[hf, :],
                                 DT[hf, :], 0.0, OP.mult, OP.add)

        # ---------------- emit fc1 + fc2/xp + scan schedule -------------
        # fc2/xp groups only need the same column-group of fc1 outputs,
        # so they interleave into fc1's window (hides the fc2 chain /
        # evac serialization).  Chunk row r reads xp cols
        # [33(r-1), 33(r-1)+63): rows 0..29 need only xp group 0, rows
        # 30..46 groups 0-1, rows 47..62 groups 1-2.
        fc1_grp(0)
        fc1_grp(1)
        fc2_mm("A", 0)
        fc2_mm("A", 1)
        xp_mm("A", 0)
        fc1_grp(2)
        xp_mm("A", 1)
        gather("A", 0, 30, nc.sync, cp=0)
        fc2_mm("A", 2)
        fc2_mm("B", 0)
        xp_mm("A", 2)
        gather("A", 30, 47, nc.gpsimd, cp=1)
        gather("A", 47, 63, nc.sync, cp=0)
        scan_prep(0, 64)                 # overlaps pass-B matmuls
        # pass-A rows of iteration 1 run while pass B is still on the PE
        scan_iter1(0, 64)
        fc2_mm("B", 1)
        fc2_mm("B", 2)
        xp_mm("B", 0)
        gather("B", 0, 30, nc.sync, cp=0)
        xp_mm("B", 1)
        gather("B", 30, 39, nc.gpsimd, cp=1)
        gather("B", 39, 47, nc.scalar, cp=0)
        xp_mm("B", 2)
        gather("B", 47, 63, nc.sync, cp=1)
        scan_prep(64, 128)
        scan_iter1(64, 128)
        scan_iter(H2T, HT, deg3=True, lo=0, hi=128)
        cur = HT

        # y = (hA - hB)/2; row r covers outputs 33*(r-1) .. +32
        hfA = cur[0:64, 1 + KW:1 + SC]
        hfB = cur[64:128, 1 + KW:1 + SC]
        nc.vector.tensor_copy(CB[0:64, :], hfB)
        nc.vector.tensor_tensor(D[0:64, :], hfA, CB[0:64, :], OP.subtract)
        nc.sync.dma_start(
            y_d.ap()[0, 0:31 * CH].rearrange("(r c) -> r c", c=CH),
            D[1:32, :])
        nc.scalar.dma_start(
            y_d.ap()[0, 31 * CH:63 * CH].rearrange("(r c) -> r c", c=CH),
            D[32:64, :])

    nc.compile()
    return nc


def _prep_inputs(inputs):
    """Host-side packing: per-core input dicts."""
    x0 = np.asarray(inputs["x0"], np.float32)[0]
    w1 = np.asarray(inputs["conv1_w"], np.float32)
    b1 = np.asarray(inputs["conv1_b"], np.float32)
    w2 = np.asarray(inputs["conv2_w"], np.float32)
    b2 = np.asarray(inputs["conv2_b"], np.float32)
    w3 = np.asarray(inputs["conv3_w"], np.float32)
    b3 = np.asarray(inputs["conv3_b"], np.float32)
    f1w = np.asarray(inputs["fc1_w"], np.float32)
    f1b = np.asarray(inputs["fc1_b"], np.float32)
    f2w = np.asarray(inputs["fc2_w"], np.float32)
    f2b = np.asarray(inputs["fc2_b"], np.float32)
    f3w = np.asarray(inputs["fc3_w"], np.float32)
    f3b = np.asarray(inputs["fc3_b"], np.float32)
    wih = np.asarray(inputs["rnn_wih"], np.float32)
    whh = np.asarray(inputs["rnn_whh"], np.float32)
    bih = np.asarray(inputs["rnn_bih"], np.float32)
    bhh = np.asarray(inputs["rnn_bhh"], np.float32)

    a = float(whh[0, 0])
    v = (wih @ f3w)[0]
    c0 = float((wih @ f3b + bih + bhh).item())

    # W1 [11, 128]: rows 0..9 conv taps, row 10 = bias (ones-row input)
    W1 = np.zeros((11, 128), np.float32)
    for c in range(2):
        for k in range(5):
            W1[c * 5 + k, 0:64] = w1[:, c, k]
            W1[c * 5 + k, 64:128] = w1[:, c, k] * (-1.0 if c == 0 else 1.0)
    W1[10, 0:64] = b1
    W1[10, 64:128] = b1

    def pack_blockdiag(w):  # (64,64,6) -> [128, 768]
        out = np.zeros((128, 768), np.float32)
        for t in range(6):
            out[0:64, 128 * t:128 * t + 64] = w[:, :, t].T
            out[64:128, 128 * t + 64:128 * t + 128] = w[:, :, t].T
        return out

    W2B = pack_blockdiag(w2)
    W3B = pack_blockdiag(w3)

    f1r = f1w.reshape(320, 64, 7)  # flat index = ch*7 + m
    F1P = np.zeros((128, 960), np.float32)
    for p in range(3):
        F1P[0:64, 320 * p:320 * p + 320] = f1r[:, :, 2 * p].T
        F1P[64:128, 320 * p:320 * p + 320] = f1r[:, :, 2 * p + 1].T
    F1S = np.zeros((64, 320), np.float32)
    F1S[:, :] = f1r[:, :, 6].T
    FB1 = np.zeros((128, 3), np.float32)
    FB1[:, 0] = f1b[0:128]
    FB1[:, 1] = f1b[128:256]
    FB1[0:64, 2] = f1b[256:320]
    FB1[64:128, 2] = f1b[256:320]

    F2 = np.zeros((128, 160), np.float32)
    F2[:, 0:80] = f2w[:, 0:128].T
    F2[:, 80:160] = f2w[:, 128:256].T
    F2S2 = np.zeros((128, 80), np.float32)
    F2S2[0:64, :] = f2w[:, 256:320].T
    F2S2[64:128, :] = f2w[:, 256:320].T
    FB2 = np.zeros((128, 1), np.float32)
    FB2[0:80, 0] = f2b
    VVc = np.zeros((128, 2), np.float32)
    VVc[0:80, 0] = v
    VVc[0:80, 1] = v

    import ml_dtypes
    bf = ml_dtypes.bfloat16
    C2 = 160 + 80 + 2
    C3 = 2 + 3 + 1 + SC + 1

    pkb = F1P.astype(bf)
    pks = F1S.astype(bf)
    F1X = np.zeros((128, 128), np.float32)
    F1X[0:64, 0:64] = F1S[:, 256:320]
    F1X[64:128, 64:128] = F1S[:, 256:320]
    pkx = F1X.astype(bf)
    pk2 = np.concatenate([F2, F2S2, VVc], axis=1)
    assert pk2.shape == (128, C2)
    W2B = W2B.astype(bf)
    W3B = W3B.astype(bf)

    B2AB = np.concatenate([b2, b2]).reshape(128, 1)
    B3AB = np.concatenate([b3, b3]).reshape(128, 1)

    QW = 810
    QC = 128 + QW
    lpad = HALO
    rpad = (7 * P - HALO + 802 * 2 + QW + 8) - L
    xpad = np.zeros((2, lpad + L + max(rpad, 0)), np.float32)
    xpad[:, lpad:lpad + L] = x0

    in_maps = []
    for core in range(8):
        s = P * core
        base = lpad + s - HALO
        # window thirds: third q rows [11q:11q+11], cols 0:128 = W1,
        # cols 128: = x windows for global conv1 sites 802q..802q+809
        xq = np.zeros((33, QC), np.float32)
        for q in range(3):
            xq[11 * q:11 * q + 11, 0:128] = W1
            qb = base + 802 * q
            for c in range(2):
                for k in range(5):
                    xq[11 * q + c * 5 + k, 128:128 + QW] = \
                        xpad[c, qb + k:qb + k + QW]
            xq[11 * q + 10, 128:128 + QW] = 1.0
        xqb = xq.astype(bf)
        # scan mask: rows 0 and 64 kill garbage; row r (1..63) col j is
        # position s - KW + CH*(r-1) + j; zero where position < 0
        mask = np.ones((128, SC), np.float32)
        mask[0, :] = 0.0
        mask[64, :] = 0.0
        if core == 0:
            for rr in range(1, 64):
                for j in range(SC):
                    if s - HALO + CH * (rr - 1) + j < 0:
                        mask[rr, j] = 0.0
                        mask[64 + rr, j] = 0.0
        pk3 = np.zeros((128, C3), np.float32)
        pk3[:, 0:1] = B2AB
        pk3[:, 1:2] = B3AB
        pk3[:, 2:5] = FB1
        pk3[:, 5:6] = FB2
        pk3[:, 6:6 + SC] = mask
        in_maps.append(dict(xq0=xqb[0:11], xq1=xqb[11:22], xq2=xqb[22:33],
                            pw2=W2B, pw3=W3B, pkb=pkb, pks=pks, pkx=pkx,
                            pk2=pk2, pk3=pk3))
    return in_maps, a, c0


LAST_RESULT = None


def kernel(**inputs) -> np.ndarray:
    global LAST_RESULT
    from concourse import bass_utils

    in_maps, a, c0 = _prep_inputs(inputs)
    nc = _build_program(a, c0)
    res = bass_utils.run_bass_kernel_spmd(nc, in_maps, core_ids=list(range(8)))
    LAST_RESULT = res
    out = np.empty((1, W), np.float32)
    for core in range(8):
        out[0, P * core:P * core + P] = res.results[core]["y"][0][:P]
    out *= 0.5
    return out


# revision 27
# speedup vs baseline: 1.0454x; 1.0454x over previous
"""Trainium2 Bass kernel for nn_Net_25847113187867 (dense_cnn).

The reference slides W = 16384 stride-1 windows over x (1,2,L), runs
conv(s5)/conv(s3)/conv(s2) + 3-layer MLP + hidden-size-1 Elman RNN per
window, twice (second pass with x channel 0 negated), and returns the
antisymmetrized scan outputs (y - y_)/2.

Restructure (final):
  * Window conv stack == dilated convs over the full sequence; fc3+RNN
    input row folded into one 80->1 vector on the host; conv1 bias
    folded into an ones-row of the input matrix.
  * Pass A and pass B (negated ch0) share one activation tile per conv
    stage ([A; B] across the 128 partitions); conv2/conv3 use
    block-diagonal weights so one matmul per tap computes both passes.
  * conv2/conv3/fc1 run fully in bf16 (weights shipped bf16 and used
    directly -- no on-device widening; activations evacuated to bf16),
    halving LDWEIGHTS time and evacuation cost.  fc2/xp stay f32r.
    Total dtype error ~8e-3 vs the 2e-2 gate.
  * Startup: SBUF DMA writes are partition-port-bound (~2GB/s per
    partition), so the 11-row conv1 window matrix ships in bf16 as 3
    column-thirds, one per DMA queue, at partition offsets 0/32/64 (PE
    weight tiles at 32-aligned rows).  conv1 evacs split across both
    act engines at an aligned 512 boundary so conv2 unblocks early.
    5 fp32 warm-up matmuls ramp the HAM clock gate while the input
    lands; 1 filler bridges c1->c2 (a PE gap drops the clock p-state).
  * fc2/xp interleave into fc1's window (same column-group of fc1
    feeds them); per pass the 3 fc2 chains run back-to-back, then the
    3 xp groups -- evacuations hide under later chains.  fc2
    accumulation steps interleave the two psum sub-blocks.
  * xp matmuls write 2 identical psum rows (VV has 2 columns), so one
    [2,gw] store lands 2 XPR copies on adjacent partitions = 2
    independent DMA-gather read ports; stores split across vector and
    scalar so gathers wait on at most 2 writer engines.
  * Chunked tanh scan: 63 rows x 33 outputs per pass, 20-step warmup
    halo, 2 DEER/Newton iterations seeded at h=0 (the seed quality is
    irrelevant after 2 iterations), so iteration 1 degenerates to
    tanh+Jacobian+scan (6 ops).  Degree-3 tanh polynomial everywhere
    (|z|<=0.22; errors correlate across the antisymmetrized passes).
    Pass-A iteration 1 runs while pass B is still on the PE; the final
    combine defers the /2 to the host.
  * Matmul column blocks all >= 256 (f32r runs 4x slower below 256),
    sub-blocks <= 512 (psum-bank limit on the matmul dst).
  * 8 cores split outputs into 2048-position slices (overlapping input
    halos, no collectives).  85.9us baseline -> 69.7us.
"""

import numpy as np

L = 16684
W = 16384
P = 2048            # output positions per core
CH = 33             # scan chunk length (output steps per chunk row)
KW = 20             # per-chunk warmup halo steps (|whh|^20 * 0.33 ~ 2e-3)
SC = KW + CH        # 53 scan columns per chunk row
HALO = KW           # left halo of xp positions per core
NY = 62 * CH + SC + 1  # 2100 xp positions per core: [s-20, s+2080)
NC3 = NY + 204      # 2304 c3 positions per core (>= NY+180, padded so
                    # _groups(NC3, balance=True) hits the [1024,x,1024] form)
NC2 = NC3 + 76      # 2380
NC1 = NC2 + 26      # 2406
NX = NC1 + 6        # 2412
SCAN_ITERS = 2
N_WARMUP = 5        # dummy fp32 matmuls to ramp the PE clock gate


def _groups(n, balance=False):
    """column groups (<=1024, psum-bank pair) with all sub-blocks in
    [256, 512], never crossing a 512-col psum bank line, everything even.
    balance=True puts the small remainder group in the MIDDLE so the
    stage ends with a full-length group whose matmuls cover the earlier
    groups' evacuation latency."""
    assert n % 2 == 0
    if balance and 2048 + 256 <= n <= 3072:
        widths = [1024, n - 2048, 1024]
    else:
        widths = []
        o = 0
        while o < n:
            rem = n - o
            gw = min(1024, rem)
            if rem > gw and rem - gw < 256:
                gw = rem - 256
            widths.append(gw)
            o += gw
    out, o = [], 0
    for gw in widths:
        if gw <= 512:
            subs = [(0, gw)]
        elif gw <= 768:
            subs = [(0, 512), (512, gw - 512)] if gw - 512 >= 256 else \
                   [(0, gw - 256), (gw - 256, 256)]
        else:
            subs = [(0, 512), (512, gw - 512)]
        out.append((o, gw, subs))
        o += gw
    return out


def _build_program(a_const, c0_const):
    import concourse.bass as bass
    import concourse.mybir as mybir
    import concourse.tile as tile
    from concourse import bacc
    from contextlib import ExitStack

    dt = mybir.dt
    f32 = dt.float32
    AF = mybir.ActivationFunctionType
    OP = mybir.AluOpType
    f32r = dt.float32r
    bf16 = dt.bfloat16

    C2 = 160 + 80 + 2              # PK2 cols (f32r): F2 F2S2 VV(pad 2)
    C3 = 2 + 3 + 1 + SC + 1        # PK3 cols: B2AB B3AB FB1 FB2 MASK (+pad)
    QW = 810                       # conv1 input cols per third (802+halo)
    QC = 128 + QW                  # third cols: W1 then windowed input

    nc = bacc.Bacc("TRN2", target_bir_lowering=False, debug=False,
                   num_devices=8)

    xq0_d = nc.dram_tensor("xq0", [11, QC], bf16, kind="ExternalInput")
    xq1_d = nc.dram_tensor("xq1", [11, QC], bf16, kind="ExternalInput")
    xq2_d = nc.dram_tensor("xq2", [11, QC], bf16, kind="ExternalInput")
    pw2_d = nc.dram_tensor("pw2", [128, 768], bf16, kind="ExternalInput")
    pw3_d = nc.dram_tensor("pw3", [128, 768], bf16, kind="ExternalInput")
    pkb_d = nc.dram_tensor("pkb", [128, 960], bf16, kind="ExternalInput")
    pks_d = nc.dram_tensor("pks", [64, 320], bf16, kind="ExternalInput")
    pk2_d = nc.dram_tensor("pk2", [128, C2], f32r, kind="ExternalInput")
    pk3_d = nc.dram_tensor("pk3", [128, C3], f32, kind="ExternalInput")
    y_d = nc.dram_tensor("y", [1, 63 * CH], f32, kind="ExternalOutput")

    with ExitStack() as ctx:
        tc = ctx.enter_context(tile.TileContext(nc))
        wp = ctx.enter_context(tc.tile_pool(name="weights", bufs=1))
        sp = ctx.enter_context(tc.tile_pool(name="acts", bufs=1))
        pp = ctx.enter_context(tc.tile_pool(name="ps", bufs=4, space="PSUM"))

        WU = wp.tile([128, 192], f32, name="WU", tag="WU")
        XWQ = wp.tile([75, QC], bf16, name="xwq", tag="xwq")
        W2B = wp.tile([128, 768], bf16, name="pw2", tag="pw2")
        W3B = wp.tile([128, 768], bf16, name="pw3", tag="pw3")
        F1P = wp.tile([128, 960], bf16, name="pkb", tag="pkb")
        F1S = wp.tile([64, 320], bf16, name="pks", tag="pks")
        PK2 = wp.tile([128, C2], f32r, name="pk2", tag="pk2")
        PK3 = wp.tile([128, C3], f32, name="pk3", tag="pk3")

        # -------- DMA issue: one input third per queue, then weights ----
        nc.vector.memset(WU[:, :], 0.0)
        nc.sync.dma_start(XWQ[0:11, :], xq0_d.ap())
        nc.scalar.dma_start(XWQ[32:43, :], xq1_d.ap())
        nc.gpsimd.dma_start(XWQ[64:75, :], xq2_d.ap())
        nc.sync.dma_start(W2B[:, 0:384], pw2_d.ap()[:, 0:384])
        nc.scalar.dma_start(W2B[:, 384:768], pw2_d.ap()[:, 384:768])
        nc.sync.dma_start(PK3[:], pk3_d.ap())
        nc.sync.dma_start(W3B[:], pw3_d.ap())
        nc.gpsimd.dma_start(F1P[:], pkb_d.ap())
        nc.sync.dma_start(PK2[:], pk2_d.ap())
        nc.sync.dma_start(F1S[:], pks_d.ap())

        F2 = PK2[:, 0:160]
        F2S2 = PK2[:, 160:240]
        VV = PK2[:, 240:242]
        B2AB = PK3[:, 0:1]
        B3AB = PK3[:, 1:2]
        FB1 = PK3[:, 2:5]
        FB2 = PK3[:, 5:6]
        MASK = PK3[:, 6:6 + SC]

        SAB = sp.tile([128, NC1], bf16, name="SAB", tag="SAB")
        TAB = sp.tile([128, NC2], bf16, name="TAB", tag="TAB")
        U_ = {"A": sp.tile([128, NC3], bf16, name="UA", tag="UA"),
              "B": sp.tile([128, NC3], bf16, name="UB", tag="UB")}
        Y1 = {("A", 0): sp.tile([128, NY], f32r, name="Y1A0", tag="Y1A0"),
              ("A", 1): sp.tile([128, NY], f32r, name="Y1A1", tag="Y1A1"),
              ("B", 0): sp.tile([128, NY], f32r, name="Y1B0", tag="Y1B0"),
              ("B", 1): sp.tile([128, NY], f32r, name="Y1B1", tag="Y1B1")}
        Y12 = sp.tile([128, NY], f32r, name="Y12", tag="Y12")
        Y2 = {"A": sp.tile([80, NY], f32r, name="Y2A", tag="Y2A"),
              "B": sp.tile([80, NY], f32r, name="Y2B", tag="Y2B")}
        XP2 = sp.tile([34, NY], f32, name="XPR", tag="XPR")
        XPR = {"A": XP2[0:2, :], "B": XP2[32:34, :]}

        # all scan scratch lives in one tile; views below carve it up
        SS = sp.tile([128, 6 * SC + 2 * (SC + 1) + 2 * CH], f32,
                     name="SS", tag="SS")
        SCT = SS[:, 0 * SC:1 * SC]
        ZT = SS[:, 1 * SC:2 * SC]
        FT = SS[:, 2 * SC:3 * SC]
        GT = SS[:, 3 * SC:4 * SC]
        DT = SS[:, 4 * SC:5 * SC]
        T2T = SS[:, 5 * SC:6 * SC]
        # cur tiles carry a zero leading column: view [:, 1:SC+1] is the
        # value, [:, 0:SC] is the shifted-by-one view
        HT = SS[:, 6 * SC:7 * SC + 1]
        H2T = SS[:, 7 * SC + 1:8 * SC + 2]
        CB = SS[:, 8 * SC + 2:8 * SC + 2 + CH]
        D = SS[:, 8 * SC + 2 + CH:8 * SC + 2 + 2 * CH]

        # zero the leading columns of the cur tiles + garbage scan rows
        nc.vector.memset(H2T[:, 0:1], 0.0)
        nc.vector.memset(SCT[0:1, :], 0.0)
        nc.vector.memset(SCT[64:65, :], 0.0)

        # ---------------- warm-up (ramps HAM clock gate) ----------------
        for i in range(N_WARMUP):
            pw = pp.tile([128, 1024], f32, name="ps", tag="ps")
            nc.tensor.matmul(pw[0:32, 0:160], WU[:, 160:192], WU[:, 0:160],
                             start=True, stop=True)

        _ct = [0]

        def evac(out_ap, ps_ap, bias_ap, eng=None):
            """relu(ps + bias) -> out; alternate scalar / vector engines."""
            if eng is None:
                use_act = _ct[0] % 2 == 0
                _ct[0] += 1
            else:
                use_act = eng == "scalar"
            if use_act:
                if bias_ap is None:
                    nc.scalar.activation(out_ap, ps_ap, AF.Relu)
                else:
                    nc.scalar.activation(out_ap, ps_ap, AF.Relu,
                                         bias=bias_ap)
            else:
                if bias_ap is None:
                    nc.vector.tensor_scalar(out_ap, ps_ap, 0.0, None, OP.max)
                else:
                    nc.vector.tensor_scalar(out_ap, ps_ap, bias_ap, 0.0,
                                            OP.add, OP.max)

        # ---------------- c1: both passes in one matmul (bf16) ----------
        # third q (PE tile rows 32q) covers global conv1 sites
        # [802q, 802q+802)
        for q in (0, 1, 2):
            rb = 32 * q
            ps = pp.tile([128, 1024], f32, name="ps", tag="ps")
            for bo, nb in ((0, 512), (512, 290)):
                nc.tensor.matmul(ps[:, bo:bo + nb],
                                 XWQ[rb:rb + 11, 0:128],
                                 XWQ[rb:rb + 11, 128 + bo:128 + bo + nb],
                                 start=True, stop=True)
            # split the evac across both engines so conv2 unblocks sooner
            evac(SAB[:, 802 * q:802 * q + 512], ps[:, 0:512], None,
                 eng="scalar")
            evac(SAB[:, 802 * q + 512:802 * q + 802], ps[:, 512:802], None,
                 eng="vector")

        # fillers: bridge c1 end -> conv2 weight DMA completion
        for i in range(1):
            pw = pp.tile([128, 1024], f32, name="ps", tag="ps")
            nc.tensor.matmul(pw[0:32, 0:160], WU[:, 160:192], WU[:, 0:160],
                             start=True, stop=True)

        # ------------- c2/c3: block-diagonal dilated convs --------------
        def conv_stage(SRC, n_out, Wt, dil):
            for goff, gw, subs in _groups(n_out, balance=True):
                ps = pp.tile([128, 1024], f32, name="ps", tag="ps")
                for t in range(6):
                    for bo, nb in subs:
                        o = goff + bo
                        nc.tensor.matmul(
                            ps[:, bo:bo + nb],
                            Wt[:, 128 * t:128 * t + 128],
                            SRC[:, o + dil * t:o + dil * t + nb],
                            start=(t == 0), stop=(t == 5))
                yield goff, gw, ps

        for goff, gw, ps in conv_stage(SAB, NC2, W2B, 5):
            evac(TAB[:, goff:goff + gw], ps[:, :gw], B2AB)

        # c3 evacuates into per-pass [site; site+30] stacks for fc1
        for goff, gw, ps in conv_stage(TAB, NC3, W3B, 15):
            for pX, rows in (("A", slice(0, 64)), ("B", slice(64, 128))):
                dst = U_[pX]
                evac(dst[0:64, goff:goff + gw], ps[rows, :gw],
                     B3AB[rows, :])
                if goff == 0:
                    evac(dst[64:128, 0:gw - 30], ps[rows, 30:gw],
                         B3AB[rows, :])
                else:
                    evac(dst[64:128, goff - 30:goff + gw - 30],
                         ps[rows, :gw], B3AB[rows, :])

        # ---------------- fc1: 448 -> 320 (tap pairs) -------------------
        gl = _groups(NY)

        def fc1_grp(gi):
            goff, gw, subs = gl[gi]
            for c in range(2):
                psc = {pX: pp.tile([128, 1024], f32, name="ps", tag="ps")
                       for pX in "AB"}
                for p in range(3):
                    wap = F1P[:, 320 * p + 128 * c:320 * p + 128 * c + 128]
                    for pX in "AB":
                        U = U_[pX]
                        for bo, nb in subs:
                            o = goff + bo
                            nc.tensor.matmul(
                                psc[pX][:, bo:bo + nb], wap,
                                U[:, o + 60 * p:o + 60 * p + nb],
                                start=(p == 0), stop=False)
                for pX in "AB":
                    U = U_[pX]
                    for bo, nb in subs:
                        o = goff + bo
                        nc.tensor.matmul(
                            psc[pX][:, bo:bo + nb],
                            F1S[0:64, 128 * c:128 * c + 128],
                            U[0:64, o + 180:o + 180 + nb],
                            start=False, stop=True)
                for pX in "AB":
                    evac(Y1[(pX, c)][:, goff:goff + gw], psc[pX][:, :gw],
                         FB1[:, c:c + 1])
            # chunk 2 (64 outs): A -> Y12[0:64], B -> Y12[64:128]
            psc = {pX: pp.tile([128, 1024], f32, name="ps", tag="ps")
                   for pX in "AB"}
            for p in range(3):
                wap = F1P[:, 320 * p + 256:320 * p + 320]
                for pX in "AB":
                    U = U_[pX]
                    for bo, nb in subs:
                        o = goff + bo
                        nc.tensor.matmul(
                            psc[pX][0:64, bo:bo + nb], wap,
                            U[:, o + 60 * p:o + 60 * p + nb],
                            start=(p == 0), stop=False)
            for pX in "AB":
                U = U_[pX]
                for bo, nb in subs:
                    o = goff + bo
                    nc.tensor.matmul(
                        psc[pX][0:64, bo:bo + nb], F1S[0:64, 256:320],
                        U[0:64, o + 180:o + 180 + nb],
                        start=False, stop=True)
            for pX, pr in (("A", 0), ("B", 64)):
                evac(Y12[pr:pr + 64, goff:goff + gw], psc[pX][0:64, :gw],
                     FB1[pr:pr + 64, 2:3])

        # ------------- fc2 + xp: pass A, then pass B --------------------
        def fc2_mm(pX, gi):
            # accumulation steps interleave the psum sub-blocks so the
            # chains pipeline; one LDW per step serves both sub-blocks.
            goff, gw, subs = gl[gi]
            ps = pp.tile([128, 1024], f32, name="ps", tag="ps")
            pr = 0 if pX == "A" else 64
            steps = ((F2[:, 0:80], Y1[(pX, 0)], slice(0, 128), 0),
                     (F2[:, 80:160], Y1[(pX, 1)], slice(0, 128), 1),
                     (F2S2[pr:pr + 64, :], Y12, slice(pr, pr + 64), 2))
            for wap, src, rows, k in steps:
                for bo, nb in subs:
                    o = goff + bo
                    nc.tensor.matmul(ps[:80, bo:bo + nb], wap,
                                     src[rows, o:o + nb],
                                     start=(k == 0), stop=(k == 2))
            evac(Y2[pX][:, goff:goff + gw], ps[:80, :gw], FB2[0:80, :],
                 eng="scalar")

        def xp_mm(pX, gi):
            # the VV stationary has 2 identical columns -> psum rows 0:2
            # are 2 copies; one [2, gw] store lands both XPR copies (on
            # adjacent partitions = 2 independent DMA-gather read ports)
            goff, gw, subs = gl[gi]
            ps2 = pp.tile([128, 1024], f32, name="ps", tag="ps")
            for bo, nb in subs:
                o = goff + bo
                nc.tensor.matmul(ps2[:2, bo:bo + nb], VV[0:80, :],
                                 Y2[pX][:, o:o + nb], start=True, stop=True)
            # A stores + the tail-critical B g1/g2 stores on vector; B g0
            # on scalar (consecutive same-engine stores keep each gather
            # at <= 2 writer semaphores)
            if pX == "A" or gi == 2:
                nc.vector.tensor_scalar(XPR[pX][0:2, goff:goff + gw],
                                        ps2[:2, :gw], float(c0_const), None,
                                        OP.add)
            else:
                h = (gw // 2) & ~1
                nc.vector.tensor_scalar(XPR[pX][0:2, goff:goff + h],
                                        ps2[:2, :h], float(c0_const), None,
                                        OP.add)
                nc.scalar.activation(XPR[pX][0:2, goff + h:goff + gw],
                                     ps2[:2, h:gw], AF.Copy,
                                     bias=float(c0_const))

        # ------------- merged A/B chunked tanh scan pieces --------------
        # rows 1:64 = pass A chunks, rows 65:128 = pass B chunks
        def ptanh(out, z, rows, deg3=False, eng=None):
            """tanh(z) for |z|<=0.35 as z*(1 - t/3 + 2t^2/15), t=z^2.
            deg3 drops the t^2 term (err ~3e-4): fine for points that
            only seed a Newton step."""
            e = eng or nc.vector
            t2 = T2T[rows, :]
            g = GT[rows, :]
            e.tensor_tensor(t2, z, z, OP.mult)
            if deg3:
                e.tensor_scalar(g, t2, -1.0 / 3.0, 1.0, OP.mult, OP.add)
                e.tensor_tensor(out, g, z, OP.mult)
                return
            e.tensor_scalar(g, t2, 2.0 / 15.0, -1.0 / 3.0, OP.mult, OP.add)
            e.tensor_tensor(g, g, t2, OP.mult)
            e.scalar_tensor_tensor(out, g, 1.0, z, OP.add, OP.mult)

        def gather(pX, r0, r1, eng, cp=0):
            # gather xp chunk rows [r0, r1) of this pass into SCT; cp
            # selects which XPR copy (partition) serves the read
            rlo = 1 if pX == "A" else 65
            xpr = XPR[pX][cp:cp + 1, :]
            src = bass.AP(tensor=xpr.tensor,
                          offset=xpr.offset + CH * r0,
                          ap=[[NY, 1], [CH, r1 - r0], [1, SC]])
            eng.dma_start(SCT[rlo + r0:rlo + r1, :], src)

        def scan_prep(lo, hi):
            # mask only: the scan seeds at h=0 (two Newton iterations wash
            # out the seed entirely; verified 1.9e-4 scan error)
            half = slice(lo, hi)
            nc.vector.tensor_tensor(SCT[half, :], SCT[half, :],
                                    MASK[half, :], OP.mult)

        def scan_iter1(lo, hi):
            # iteration 1 specialized for the zero seed: Z == sct and
            # d1 == F, so it is just tanh, the Jacobian, and the scan
            hf = slice(lo, hi)
            ptanh(FT[hf, :], SCT[hf, :], hf, deg3=True)
            nc.vector.tensor_tensor(GT[hf, :], FT[hf, :], FT[hf, :], OP.mult)
            nc.vector.tensor_scalar(GT[hf, :], GT[hf, :], -a, a, OP.mult,
                                    OP.add)
            nc.vector.tensor_tensor_scan(H2T[hf, 1:SC + 1], GT[hf, :],
                                         FT[hf, :], 0.0, OP.mult, OP.add)

        a = float(a_const)

        def scan_iter(cur, nxt, deg3, lo=0, hi=128, eng=None):
            e = eng or nc.vector
            hf = slice(lo, hi)
            # Z = a*cur_shifted + sct  (leading zero col makes col0 = sct0)
            if e is nc.vector:
                e.scalar_tensor_tensor(ZT[hf, :], cur[hf, 0:SC], a,
                                       SCT[hf, :], OP.mult, OP.add)
            else:
                # gpsimd has no scalar_tensor_tensor
                e.tensor_scalar(ZT[hf, :], cur[hf, 0:SC], a, None, OP.mult)
                e.tensor_tensor(ZT[hf, :], ZT[hf, :], SCT[hf, :], OP.add)
            ptanh(FT[hf, :], ZT[hf, :], hf, deg3=deg3, eng=e)
            # G = a*(1 - F^2)
            e.tensor_tensor(GT[hf, :], FT[hf, :], FT[hf, :], OP.mult)
            e.tensor_scalar(GT[hf, :], GT[hf, :], -a, a, OP.mult,
                            OP.add)
            # d1 = F - G*cur_shifted;  nxt_t = G_t*nxt_{t-1} + d1_t
            e.tensor_tensor(T2T[hf, :], GT[hf, :], cur[hf, 0:SC],
                            OP.mult)
            e.tensor_tensor(DT[hf, :], FT[hf, :], T2T[hf, :],
                            OP.subtract)
            e.tensor_tensor_scan(nxt[hf, 1:SC + 1], GT[hf, :],
                                 DT[hf, :], 0.0, OP.mult, OP.add)

        # ---------------- emit fc1 + fc2/xp + scan schedule -------------
        # fc2/xp groups only need the same column-group of fc1 outputs,
        # so they interleave into fc1's window (hides the fc2 chain /
        # evac serialization).  Chunk row r reads xp cols
        # [33(r-1), 33(r-1)+63): rows 0..29 need only xp group 0, rows
        # 30..46 groups 0-1, rows 47..62 groups 1-2.
        fc1_grp(0)
        fc1_grp(1)
        fc2_mm("A", 0)
        fc2_mm("A", 1)
        xp_mm("A", 0)
        fc1_grp(2)
        xp_mm("A", 1)
        gather("A", 0, 30, nc.sync, cp=0)
        fc2_mm("A", 2)
        fc2_mm("B", 0)
        xp_mm("A", 2)
        gather("A", 30, 47, nc.gpsimd, cp=1)
        gather("A", 47, 63, nc.sync, cp=0)
        scan_prep(0, 64)                 # overlaps pass-B matmuls
        # pass-A rows of iteration 1 run while pass B is still on the PE
        scan_iter1(0, 64)
        fc2_mm("B", 1)
        fc2_mm("B", 2)
        xp_mm("B", 0)
        gather("B", 0, 30, nc.sync, cp=0)
        xp_mm("B", 1)
        gather("B", 30, 39, nc.gpsimd, cp=1)
        gather("B", 39, 47, nc.scalar, cp=0)
        xp_mm("B", 2)
        gather("B", 47, 63, nc.sync, cp=1)
        scan_prep(64, 128)
        scan_iter1(64, 128)
        scan_iter(H2T, HT, deg3=True, lo=0, hi=128)
        cur = HT

        # y = (hA - hB)/2; row r covers outputs 33*(r-1) .. +32
        hfA = cur[0:64, 1 + KW:1 + SC]
        hfB = cur[64:128, 1 + KW:1 + SC]
        nc.vector.tensor_copy(CB[0:64, :], hfB)
        nc.vector.tensor_tensor(D[0:64, :], hfA, CB[0:64, :], OP.subtract)
        nc.sync.dma_start(
            y_d.ap()[0, 0:31 * CH].rearrange("(r c) -> r c", c=CH),
            D[1:32, :])
        nc.scalar.dma_start(
            y_d.ap()[0, 31 * CH:63 * CH].rearrange("(r c) -> r c", c=CH),
            D[32:64, :])

    nc.compile()
    return nc


def _prep_inputs(inputs):
    """Host-side packing: per-core input dicts."""
    x0 = np.asarray(inputs["x0"], np.float32)[0]
    w1 = np.asarray(inputs["conv1_w"], np.float32)
    b1 = np.asarray(inputs["conv1_b"], np.float32)
    w2 = np.asarray(inputs["conv2_w"], np.float32)
    b2 = np.asarray(inputs["conv2_b"], np.float32)
    w3 = np.asarray(inputs["conv3_w"], np.float32)
    b3 = np.asarray(inputs["conv3_b"], np.float32)
    f1w = np.asarray(inputs["fc1_w"], np.float32)
    f1b = np.asarray(inputs["fc1_b"], np.float32)
    f2w = np.asarray(inputs["fc2_w"], np.float32)
    f2b = np.asarray(inputs["fc2_b"], np.float32)
    f3w = np.asarray(inputs["fc3_w"], np.float32)
    f3b = np.asarray(inputs["fc3_b"], np.float32)
    wih = np.asarray(inputs["rnn_wih"], np.float32)
    whh = np.asarray(inputs["rnn_whh"], np.float32)
    bih = np.asarray(inputs["rnn_bih"], np.float32)
    bhh = np.asarray(inputs["rnn_bhh"], np.float32)

    a = float(whh[0, 0])
    v = (wih @ f3w)[0]
    c0 = float((wih @ f3b + bih + bhh).item())

    # W1 [11, 128]: rows 0..9 conv taps, row 10 = bias (ones-row input)
    W1 = np.zeros((11, 128), np.float32)
    for c in range(2):
        for k in range(5):
            W1[c * 5 + k, 0:64] = w1[:, c, k]
            W1[c * 5 + k, 64:128] = w1[:, c, k] * (-1.0 if c == 0 else 1.0)
    W1[10, 0:64] = b1
    W1[10, 64:128] = b1

    def pack_blockdiag(w):  # (64,64,6) -> [128, 768]
        out = np.zeros((128, 768), np.float32)
        for t in range(6):
            out[0:64, 128 * t:128 * t + 64] = w[:, :, t].T
            out[64:128, 128 * t + 64:128 * t + 128] = w[:, :, t].T
        return out

    W2B = pack_blockdiag(w2)
    W3B = pack_blockdiag(w3)

    f1r = f1w.reshape(320, 64, 7)  # flat index = ch*7 + m
    F1P = np.zeros((128, 960), np.float32)
    for p in range(3):
        F1P[0:64, 320 * p:320 * p + 320] = f1r[:, :, 2 * p].T
        F1P[64:128, 320 * p:320 * p + 320] = f1r[:, :, 2 * p + 1].T
    F1S = np.zeros((64, 320), np.float32)
    F1S[:, :] = f1r[:, :, 6].T
    FB1 = np.zeros((128, 3), np.float32)
    FB1[:, 0] = f1b[0:128]
    FB1[:, 1] = f1b[128:256]
    FB1[0:64, 2] = f1b[256:320]
    FB1[64:128, 2] = f1b[256:320]

    F2 = np.zeros((128, 160), np.float32)
    F2[:, 0:80] = f2w[:, 0:128].T
    F2[:, 80:160] = f2w[:, 128:256].T
    F2S2 = np.zeros((128, 80), np.float32)
    F2S2[0:64, :] = f2w[:, 256:320].T
    F2S2[64:128, :] = f2w[:, 256:320].T
    FB2 = np.zeros((128, 1), np.float32)
    FB2[0:80, 0] = f2b
    VVc = np.zeros((128, 2), np.float32)
    VVc[0:80, 0] = v
    VVc[0:80, 1] = v

    import ml_dtypes
    bf = ml_dtypes.bfloat16
    C2 = 160 + 80 + 2
    C3 = 2 + 3 + 1 + SC + 1

    pkb = F1P.astype(bf)
    pks = F1S.astype(bf)
    pk2 = np.concatenate([F2, F2S2, VVc], axis=1)
    assert pk2.shape == (128, C2)
    W2B = W2B.astype(bf)
    W3B = W3B.astype(bf)

    B2AB = np.concatenate([b2, b2]).reshape(128, 1)
    B3AB = np.concatenate([b3, b3]).reshape(128, 1)

    QW = 810
    QC = 128 + QW
    lpad = HALO
    rpad = (7 * P - HALO + 802 * 2 + QW + 8) - L
    xpad = np.zeros((2, lpad + L + max(rpad, 0)), np.float32)
    xpad[:, lpad:lpad + L] = x0

    in_maps = []
    for core in range(8):
        s = P * core
        base = lpad + s - HALO
        # window thirds: third q rows [11q:11q+11], cols 0:128 = W1,
        # cols 128: = x windows for global conv1 sites 802q..802q+809
        xq = np.zeros((33, QC), np.float32)
        for q in range(3):
            xq[11 * q:11 * q + 11, 0:128] = W1
            qb = base + 802 * q
            for c in range(2):
                for k in range(5):
                    xq[11 * q + c * 5 + k, 128:128 + QW] = \
                        xpad[c, qb + k:qb + k + QW]
            xq[11 * q + 10, 128:128 + QW] = 1.0
        xqb = xq.astype(bf)
        # scan mask: rows 0 and 64 kill garbage; row r (1..63) col j is
        # position s - KW + CH*(r-1) + j; zero where position < 0
        mask = np.ones((128, SC), np.float32)
        mask[0, :] = 0.0
        mask[64, :] = 0.0
        if core == 0:
            for rr in range(1, 64):
                for j in range(SC):
                    if s - HALO + CH * (rr - 1) + j < 0:
                        mask[rr, j] = 0.0
                        mask[64 + rr, j] = 0.0
        pk3 = np.zeros((128, C3), np.float32)
        pk3[:, 0:1] = B2AB
        pk3[:, 1:2] = B3AB
        pk3[:, 2:5] = FB1
        pk3[:, 5:6] = FB2
        pk3[:, 6:6 + SC] = mask
        in_maps.append(dict(xq0=xqb[0:11], xq1=xqb[11:22], xq2=xqb[22:33],
                            pw2=W2B, pw3=W3B, pkb=pkb, pks=pks, pk2=pk2,
                            pk3=pk3))
    return in_maps, a, c0


LAST_RESULT = None


def kernel(**inputs) -> np.ndarray:
    global LAST_RESULT
    from concourse import bass_utils

    in_maps, a, c0 = _prep_inputs(inputs)
    nc = _build_program(a, c0)
    res = bass_utils.run_bass_kernel_spmd(nc, in_maps, core_ids=list(range(8)))
    LAST_RESULT = res
    out = np.empty((1, W), np.float32)
    for core in range(8):
        out[0, P * core:P * core + P] = res.results[core]["y"][0][:P]
    out *= 0.5
    return out
